# revision 27
# baseline (speedup 1.0000x reference)
"""Trainium2 Bass kernel for the fused broadcast multiply-add:

    out[s, i, f] = x[s, i] * W[i, f] + b[i, f]

Shapes (hardcoded): x [16384, 32] f32, W [32, 256] f32, b [32, 256] f32,
out [16384, 32, 256] f32 (512 MB) -- a pure HBM-write-bound problem.

Strategy
--------
Data parallel over 8 NeuronCores: each core handles 2048 batch rows and
writes a 64 MB output shard (~150-180 us at the measured 360-427 GB/s
per-core store bandwidth).

On each core everything is folded into TensorE matmuls. Each 512-column
output chunk n covers i = {2n, 2n+1} only, so its contraction needs just
K=8 rows (fp16 hi/lo split of x and W for full-rate PE with ~fp32
accuracy, bias via ones-rows):

    rows: x_hi[i0], x_hi[i1], x_hi[i0], x_hi[i1], x_lo[i0], x_lo[i1], 1, 1
    rhs:  W_hi[i0]|0, 0|W_hi[i1], W_lo[i0]|0, 0|W_lo[i1],
          W_hi[i0]|0, 0|W_hi[i1], b_hi, b_lo

(x*W = x_hi*W_hi + x_hi*W_lo + x_lo*W_hi; the dropped x_lo*W_lo term is
~2^-21 relative.) Consecutive chunks rotate tile_position across the four
32-row PE groups, so each matmul's LDWEIGHTS targets rows disjoint from
the in-flight matmul and the PE pipelines back-to-back instead of paying
the isolated fill+drain per instruction.

The xap activation tensor loads as four per-slot DMAs so each chunk's
matmuls only wait for their own slot (better startup overlap than one
monolithic load -- measured). PSUM accumulates fp32; VectorE/ScalarE
alternate on [128,1024] PSUM->SBUF copies; the sync-engine HWDGE streams
2 MB half-tiles to HBM. PE and the copy engines run well under the DMA
roofline, so the kernel is output-DMA-bound as the memory target_regime
intends.
"""

import numpy as np

import concourse.bass as bass
import concourse.bacc as bacc
import concourse.mybir as mybir
import concourse.tile as tile
from concourse import bass_utils

BS, DEMO, FEAT = 16384, 32, 256
NCORES = 8
BSH = BS // NCORES        # 2048 batch rows per core
PT = 128                  # batch rows per matmul tile (out partitions)
NTILES = BSH // PT        # 16
NF = DEMO * FEAT          # 8192 output columns
NCHUNK = 512              # fp32 columns per PSUM bank / matmul
NCH = NF // NCHUNK        # 16 chunks (each covers two i values)
NSLOT = NCH // 4          # 4 free-dim slots per row-group

_cache: dict = {}


def _build():
    nc = bacc.Bacc("TRN2", target_bir_lowering=False, debug=False)

    # comb: [32, NSLOT*BSH + NSLOT*NCHUNK] fp16 compact combined input --
    # row 8r+k holds row k of group r (8 lhsT rows for chunks n%4==r);
    # cols [0, NSLOT*BSH) are activation slots, the rest W/bias rhs slots.
    # Loading only the 32 used rows (0.65 MB vs 2.5 MB padded-to-128)
    # keeps load bytes OFF engines 8-15 entirely and halves them on 0-7:
    # on cores where one SDMA engine runs slow (the max-core cores), its
    # load share is what's cut, directly shortening the critical engine.
    CW = NSLOT * BSH + NSLOT * NCHUNK
    WOFF = NSLOT * BSH
    comb_d = nc.dram_tensor("comb", (32, CW), mybir.dt.float16, kind="ExternalInput")
    out_d = nc.dram_tensor("out", (BSH, NF), mybir.dt.float32, kind="ExternalOutput")

    with tile.TileContext(nc) as tc:
        with (
            tc.tile_pool(name="const", bufs=1) as cpool,
            tc.tile_pool(name="opool", bufs=3) as opool,
            tc.tile_pool(name="psum", bufs=4, space=bass.MemorySpace.PSUM) as psum,
        ):
            comb_t = cpool.tile([128, CW], mybir.dt.float16)
            for r in range(4):
                nc.sync.dma_start(
                    comb_t[32 * r:32 * r + 8, :], comb_d.ap()[8 * r:8 * r + 8, :]
                )

            for t in range(NTILES):
                o_t = opool.tile([PT, NF], mybir.dt.float32)
                for g in range(8):  # copy groups of 1024 cols (2 chunks)
                    acc = psum.tile([PT, 2 * NCHUNK], mybir.dt.float32)
                    for h in range(2):
                        n = 2 * g + h
                        r, s = n % 4, n // 4
                        nc.tensor.matmul(
                            acc[:, h * NCHUNK:(h + 1) * NCHUNK],
                            comb_t[32 * r:32 * r + 8,
                                   s * BSH + t * PT: s * BSH + (t + 1) * PT],
                            comb_t[32 * r:32 * r + 8,
                                   WOFF + s * NCHUNK: WOFF + (s + 1) * NCHUNK],
                            start=True,
                            stop=True,
                            tile_position=(32 * r, 0),
                        )
                    dst = o_t[:, g * 1024:(g + 1) * 1024]
                    if g % 2 == 0:
                        nc.vector.tensor_copy(dst, acc[:])
                    else:
                        nc.scalar.copy(dst, acc[:])
                    if g in (3, 7):  # 2 MB half-tile stores
                        lo, hi = (g - 3) * 1024, (g + 1) * 1024
                        nc.sync.dma_start(
                            out_d.ap()[t * PT:(t + 1) * PT, lo:hi],
                            o_t[:, lo:hi],
                        )

    nc.compile()
    return nc


def _get_nc():
    if "nc" not in _cache:
        _cache["nc"] = _build()
    return _cache["nc"]


def _prep(x, W, b):
    """Host-side layout prep: fp16 hi/lo split into row-group layout."""
    x = np.asarray(x, dtype=np.float32)
    W = np.asarray(W, dtype=np.float32)
    b = np.asarray(b, dtype=np.float32)

    xT = np.ascontiguousarray(x.T)                       # [DEMO, BS]
    x_hi = xT.astype(np.float16)
    x_lo = (xT - x_hi.astype(np.float32)).astype(np.float16)
    W_hi = W.astype(np.float16)
    W_lo = (W - W_hi.astype(np.float32)).astype(np.float16)
    b_hi = b.astype(np.float16)
    b_lo = (b - b_hi.astype(np.float32)).astype(np.float16)

    xap = np.zeros((128, NSLOT * BS), dtype=np.float16)
    wbp = np.zeros((128, NSLOT * NCHUNK), dtype=np.float16)
    for n in range(NCH):
        r, s = n % 4, n // 4
        i0, i1 = 2 * n, 2 * n + 1
        p = 32 * r
        xs = slice(s * BS, (s + 1) * BS)
        xap[p + 0, xs] = x_hi[i0]
        xap[p + 1, xs] = x_hi[i1]
        xap[p + 2, xs] = x_hi[i0]
        xap[p + 3, xs] = x_hi[i1]
        xap[p + 4, xs] = x_lo[i0]
        xap[p + 5, xs] = x_lo[i1]
        xap[p + 6, xs] = 1.0
        xap[p + 7, xs] = 1.0

        c0 = s * NCHUNK
        wbp[p + 0, c0:c0 + FEAT] = W_hi[i0]
        wbp[p + 1, c0 + FEAT:c0 + 2 * FEAT] = W_hi[i1]
        wbp[p + 2, c0:c0 + FEAT] = W_lo[i0]
        wbp[p + 3, c0 + FEAT:c0 + 2 * FEAT] = W_lo[i1]
        wbp[p + 4, c0:c0 + FEAT] = W_hi[i0]
        wbp[p + 5, c0 + FEAT:c0 + 2 * FEAT] = W_hi[i1]
        wbp[p + 6, c0:c0 + FEAT] = b_hi[i0]
        wbp[p + 6, c0 + FEAT:c0 + 2 * FEAT] = b_hi[i1]
        wbp[p + 7, c0:c0 + FEAT] = b_lo[i0]
        wbp[p + 7, c0 + FEAT:c0 + 2 * FEAT] = b_lo[i1]
    return xap, wbp


def _in_maps(x, W, b):
    xap, wbp = _prep(x, W, b)
    rows = np.concatenate([np.arange(32 * r, 32 * r + 8) for r in range(4)])
    maps = []
    for c in range(NCORES):
        # compact per-core shard: only the 32 used rows, xap slot columns
        # c*BSH:(c+1)*BSH with the wbp rhs slots appended on the free dim
        comb = np.empty((32, NSLOT * BSH + NSLOT * NCHUNK), dtype=np.float16)
        for s in range(NSLOT):
            comb[:, s * BSH:(s + 1) * BSH] = (
                xap[rows, s * BS + c * BSH: s * BS + (c + 1) * BSH]
            )
        comb[:, NSLOT * BSH:] = wbp[rows]
        maps.append({"comb": comb})
    return maps


def run_shards(x, W, b, **spmd_kwargs):
    """Run the SPMD kernel; returns the BassKernelResults (for profiling)."""
    nc = _get_nc()
    return bass_utils.run_bass_kernel_spmd(
        nc, _in_maps(x, W, b), core_ids=list(range(NCORES)), **spmd_kwargs
    )


def kernel(x, W, b):
    res = run_shards(x, W, b)
    out = np.concatenate([res.results[c]["out"] for c in range(NCORES)], axis=0)
    return out.reshape(BS, DEMO, FEAT)



# revision 29
# speedup vs baseline: 1.1259x; 1.1259x over previous
"""Trainium2 Bass kernel for the fused broadcast multiply-add:

    out[s, i, f] = x[s, i] * W[i, f] + b[i, f]

Shapes (hardcoded): x [16384, 32] f32, W [32, 256] f32, b [32, 256] f32,
out [16384, 32, 256] f32 (512 MB) -- a pure HBM-write-bound problem.

Strategy
--------
Data parallel over 8 NeuronCores: each core handles 2048 batch rows and
writes a 64 MB output shard (~150-180 us at the measured 360-427 GB/s
per-core store bandwidth).

On each core everything is folded into TensorE matmuls. Each 512-column
output chunk n covers i = {2n, 2n+1} only, so its contraction needs just
K=8 rows (fp16 hi/lo split of x and W for full-rate PE with ~fp32
accuracy, bias via ones-rows):

    rows: x_hi[i0], x_hi[i1], x_hi[i0], x_hi[i1], x_lo[i0], x_lo[i1], 1, 1
    rhs:  W_hi[i0]|0, 0|W_hi[i1], W_lo[i0]|0, 0|W_lo[i1],
          W_hi[i0]|0, 0|W_hi[i1], b_hi, b_lo

(x*W = x_hi*W_hi + x_hi*W_lo + x_lo*W_hi; the dropped x_lo*W_lo term is
~2^-21 relative.) Consecutive chunks rotate tile_position across the four
32-row PE groups, so each matmul's LDWEIGHTS targets rows disjoint from
the in-flight matmul and the PE pipelines back-to-back instead of paying
the isolated fill+drain per instruction.

The xap activation tensor loads as four per-slot DMAs so each chunk's
matmuls only wait for their own slot (better startup overlap than one
monolithic load -- measured). PSUM accumulates fp32; VectorE/ScalarE
alternate on [128,1024] PSUM->SBUF copies; the sync-engine HWDGE streams
2 MB half-tiles to HBM. PE and the copy engines run well under the DMA
roofline, so the kernel is output-DMA-bound as the memory target_regime
intends.
"""

import numpy as np

import concourse.bass as bass
import concourse.bacc as bacc
import concourse.mybir as mybir
import concourse.tile as tile
from concourse import bass_utils

BS, DEMO, FEAT = 16384, 32, 256
NCORES = 8
BSH = BS // NCORES        # 2048 batch rows per core
PT = 128                  # batch rows per matmul tile (out partitions)
NTILES = BSH // PT        # 16
NF = DEMO * FEAT          # 8192 output columns
NCHUNK = 512              # fp32 columns per PSUM bank / matmul
NCH = NF // NCHUNK        # 16 chunks (each covers two i values)
NSLOT = NCH // 4          # 4 free-dim slots per row-group

_cache: dict = {}


def _build():
    nc = bacc.Bacc("TRN2", target_bir_lowering=False, debug=False)

    # xap: [128, NSLOT*BSH] fp16 -- row-group r holds the 8 lhsT rows for
    # chunks n with n%4==r, at free offset (n//4)*BSH.
    # wbp: [128, NSLOT*NCHUNK] fp16 -- same layout for the rhs slices.
    xap_d = nc.dram_tensor(
        "xap", (128, NSLOT * BSH), mybir.dt.float16, kind="ExternalInput"
    )
    wbp_d = nc.dram_tensor(
        "wbp", (128, NSLOT * NCHUNK), mybir.dt.float16, kind="ExternalInput"
    )
    out_d = nc.dram_tensor("out", (BSH, NF), mybir.dt.float32, kind="ExternalOutput")

    with tile.TileContext(nc) as tc:
        with (
            tc.tile_pool(name="const", bufs=1) as cpool,
            tc.tile_pool(name="opool", bufs=3) as opool,
            tc.tile_pool(name="psum", bufs=4, space=bass.MemorySpace.PSUM) as psum,
        ):
            wbp_t = cpool.tile([128, NSLOT * NCHUNK], mybir.dt.float16)
            xap_t = cpool.tile([128, NSLOT * BSH], mybir.dt.float16)
            nc.sync.dma_start(wbp_t[:], wbp_d.ap()[:])
            # split the xap load by slot so the first chunks start early
            for s in range(NSLOT):
                nc.sync.dma_start(
                    xap_t[:, s * BSH:(s + 1) * BSH],
                    xap_d.ap()[:, s * BSH:(s + 1) * BSH],
                )

            for t in range(NTILES):
                o_t = opool.tile([PT, NF], mybir.dt.float32)
                for g in range(8):  # copy groups of 1024 cols (2 chunks)
                    acc = psum.tile([PT, 2 * NCHUNK], mybir.dt.float32)
                    for h in range(2):
                        n = 2 * g + h
                        r, s = n % 4, n // 4
                        nc.tensor.matmul(
                            acc[:, h * NCHUNK:(h + 1) * NCHUNK],
                            xap_t[32 * r:32 * r + 8,
                                  s * BSH + t * PT: s * BSH + (t + 1) * PT],
                            wbp_t[32 * r:32 * r + 8,
                                  s * NCHUNK:(s + 1) * NCHUNK],
                            start=True,
                            stop=True,
                            tile_position=(32 * r, 0),
                        )
                    dst = o_t[:, g * 1024:(g + 1) * 1024]
                    if g % 2 == 0:
                        nc.vector.tensor_copy(dst, acc[:])
                    else:
                        nc.scalar.copy(dst, acc[:])
                    if g in (3, 7):  # 2 MB half-tile stores, alternating
                        # between the two HWDGE rings (SP and ACT) to halve
                        # per-ring instruction/descriptor-fetch traffic
                        lo, hi = (g - 3) * 1024, (g + 1) * 1024
                        dma_eng = nc.sync if g == 3 else nc.scalar
                        dma_eng.dma_start(
                            out_d.ap()[t * PT:(t + 1) * PT, lo:hi],
                            o_t[:, lo:hi],
                        )

    nc.compile()
    return nc


def _get_nc():
    if "nc" not in _cache:
        _cache["nc"] = _build()
    return _cache["nc"]


def _prep(x, W, b):
    """Host-side layout prep: fp16 hi/lo split into row-group layout."""
    x = np.asarray(x, dtype=np.float32)
    W = np.asarray(W, dtype=np.float32)
    b = np.asarray(b, dtype=np.float32)

    xT = np.ascontiguousarray(x.T)                       # [DEMO, BS]
    x_hi = xT.astype(np.float16)
    x_lo = (xT - x_hi.astype(np.float32)).astype(np.float16)
    W_hi = W.astype(np.float16)
    W_lo = (W - W_hi.astype(np.float32)).astype(np.float16)
    b_hi = b.astype(np.float16)
    b_lo = (b - b_hi.astype(np.float32)).astype(np.float16)

    xap = np.zeros((128, NSLOT * BS), dtype=np.float16)
    wbp = np.zeros((128, NSLOT * NCHUNK), dtype=np.float16)
    for n in range(NCH):
        r, s = n % 4, n // 4
        i0, i1 = 2 * n, 2 * n + 1
        p = 32 * r
        xs = slice(s * BS, (s + 1) * BS)
        xap[p + 0, xs] = x_hi[i0]
        xap[p + 1, xs] = x_hi[i1]
        xap[p + 2, xs] = x_hi[i0]
        xap[p + 3, xs] = x_hi[i1]
        xap[p + 4, xs] = x_lo[i0]
        xap[p + 5, xs] = x_lo[i1]
        xap[p + 6, xs] = 1.0
        xap[p + 7, xs] = 1.0

        c0 = s * NCHUNK
        wbp[p + 0, c0:c0 + FEAT] = W_hi[i0]
        wbp[p + 1, c0 + FEAT:c0 + 2 * FEAT] = W_hi[i1]
        wbp[p + 2, c0:c0 + FEAT] = W_lo[i0]
        wbp[p + 3, c0 + FEAT:c0 + 2 * FEAT] = W_lo[i1]
        wbp[p + 4, c0:c0 + FEAT] = W_hi[i0]
        wbp[p + 5, c0 + FEAT:c0 + 2 * FEAT] = W_hi[i1]
        wbp[p + 6, c0:c0 + FEAT] = b_hi[i0]
        wbp[p + 6, c0 + FEAT:c0 + 2 * FEAT] = b_hi[i1]
        wbp[p + 7, c0:c0 + FEAT] = b_lo[i0]
        wbp[p + 7, c0 + FEAT:c0 + 2 * FEAT] = b_lo[i1]
    return xap, wbp


def _in_maps(x, W, b):
    xap, wbp = _prep(x, W, b)
    maps = []
    for c in range(NCORES):
        # per-core xap shard: batch columns c*BSH:(c+1)*BSH of each slot
        shard = np.empty((128, NSLOT * BSH), dtype=np.float16)
        for s in range(NSLOT):
            shard[:, s * BSH:(s + 1) * BSH] = (
                xap[:, s * BS + c * BSH: s * BS + (c + 1) * BSH]
            )
        maps.append({"xap": shard, "wbp": wbp})
    return maps


def run_shards(x, W, b, **spmd_kwargs):
    """Run the SPMD kernel; returns the BassKernelResults (for profiling)."""
    nc = _get_nc()
    return bass_utils.run_bass_kernel_spmd(
        nc, _in_maps(x, W, b), core_ids=list(range(NCORES)), **spmd_kwargs
    )


def kernel(x, W, b):
    res = run_shards(x, W, b)
    out = np.concatenate([res.results[c]["out"] for c in range(NCORES)], axis=0)
    return out.reshape(BS, DEMO, FEAT)

